# revision 20
# baseline (speedup 1.0000x reference)
"""Trainium2 Bass kernel for nn_Attention_10711648436709.

Math (faithful to reference):
    h = einsum('bhik,bhjk->bhij', Q, K) / sqrt(H)     # scale = sqrt(16) = 4
    w = softmax(h, axis=0)                            # over the BATCH axis (B=4)
    out = einsum('bhij,bhjv->bhiv', w, V)
    (mask is a no-op in the reference)

Sharding: head-parallel across 8 cores (16 heads -> 2 heads/core).
Softmax over batch stays core-local => communication-free.

Per-core layout trick: compute transposed scores S^T[j, i] so that
 - QK:  lhsT = K^T[d, j-block]  rhs = Q^T[d, i-chunk]   (host pre-transposes Q,K)
 - PV:  lhsT = V[j-block, v]    rhs = W[j, i-chunk]     (V in natural layout)
 - output accumulates as out^T[v, i] in PSUM; host transposes back.

Batch-0-pivot softmax: g_b = h_b - h_0 (b=1..3) computed by ONE full-K=128
matmul each (lhsT = [K_b^T ; K_0^T], rhs = [Q_b^T ; -Q_0^T], host packs).
Then w_b = E_b * r with E_b = e^{g_b/4}, r = 1/(1 + E_1 + E_2 + E_3), and
w_0 = r.  The whole denominator+reciprocal runs as ONE custom-DVE op
(r = 1/(1 + in0 + in1), BITWISE_NOT exponent-flip seed + 1 Newton pass,
max rel err ~2e-3) instead of the previous 5-op Newton / ACT Ln-Exp chain.
"""

import sys
import os

for p in ("/opt/trn_rl_repo",):
    if p not in sys.path:
        sys.path.insert(0, p)

import numpy as np
import ml_dtypes

B, H, S, D = 4, 16, 2048, 64
NCORES = 8
HL = H // NCORES          # 2 heads per core
NB = S // 128             # 16 j-blocks
NI = S // 512             # 4 i-chunks

TRACE = False
LAST_EXEC_NS = None
LAST_RESULTS = None

_NC = None
_RECIP_OP = None

# Chebyshev-minimax seed constants from RECIP_APPROX_FAST_CONSTS
_RC0 = -0.23549792
_RC1 = 2.0017324


def _register_recip1p():
    """Register the fused custom-DVE op  out = 1/(1 + in0 + in1).

    d = in0 + in1 + 1; seed = bitcast(~bits(d)) * c0  (exponent-flip trick,
    d > 1 always so the seed interval [-4.5,-4] for d*~d holds); one inline
    Newton pass y0*(c1 - d*y0).  7 ALU stages (<= 8 budget)."""
    global _RECIP_OP
    if _RECIP_OP is not None:
        return _RECIP_OP
    import concourse.dve_ops as dvo
    from concourse.dve_spec import (
        Spec,
        Src0,
        Src1,
        C0,
        C1,
        One,
        Bin,
        AluOp,
        lower,
        _has_src1 as has_src1,
    )
    from concourse.dve_uop import DveOpSpec

    NAME = "RECIP1P_ANT"
    if NAME in dvo._SUB_OPCODE_FOR_NAME:
        _RECIP_OP = next(o for o in dvo.OPS if o.name == NAME)
        return _RECIP_OP

    d = (Src0 + Src1) + One
    nx = Bin(AluOp.BITWISE_NOT, d, d)
    y0 = nx * C0
    y1 = y0 * (C1 - d * y0)

    def _ref(in0, in1, s0, s1, imm2):
        dd = in0.astype(np.float32) + in1.astype(np.float32) + np.float32(1.0)
        nxx = (~dd.view(np.int32)).view(np.float32)
        yy0 = nxx * np.float32(s0)
        return yy0 * (np.float32(s1) - dd * yy0)

    spec = Spec(body=y1, reference=_ref)
    row = max(dvo._SUB_OPCODE_FOR_NAME.values()) + 1
    assert row < 0x20, "custom-DVE opcode rows exhausted"
    shas = {}
    for ver in ("v3", "v4"):
        try:
            uops = lower(spec, ver=ver)
            shas[ver] = DveOpSpec(
                name=NAME, opcode=row, uops=uops, rd1_en=has_src1(spec)
            ).sha(ver)
        except Exception:
            pass
    op = dvo.DveOp(NAME, spec, subdim=False, uops_sha=shas)
    dvo.OPS.append(op)
    dvo.CUSTOM_DVE_SPECS[NAME] = spec
    dvo._SUB_OPCODE_FOR_NAME[NAME] = row
    _RECIP_OP = op
    return op


def _build_nc():
    import concourse.bass as bass
    import concourse.mybir as mybir
    import concourse.tile as tile

    DT = mybir.dt
    AF = mybir.ActivationFunctionType

    recip_op = _register_recip1p()

    nc = bass.Bass()
    ALU = mybir.AluOpType
    qt = nc.declare_dram_parameter("qt", [3, HL, 128, S], DT.bfloat16, isOutput=False)
    kt = nc.declare_dram_parameter("kt", [3, HL, 128, S], DT.bfloat16, isOutput=False)
    # host pre-swizzles V to [128, NB*D] per (b,hl) so the load is contiguous
    vv = nc.declare_dram_parameter(
        "v", [B, HL, 128, NB * D], DT.bfloat16, isOutput=False
    )
    out = nc.declare_dram_parameter("out", [B, HL, D, S], DT.float32, isOutput=True)

    with tile.TileContext(nc) as tc:
        with (
            tc.tile_pool(name="inputs", bufs=1) as ipool,
            tc.tile_pool(name="work", bufs=8) as wpool,
            tc.tile_pool(name="outsb", bufs=4) as opool,
            tc.tile_pool(name="qkps", bufs=2, space="PSUM") as qkpool,
            tc.tile_pool(name="ops", bufs=1, space="PSUM") as opsum,
        ):
            QT = ipool.tile([128, 3 * HL * S], DT.bfloat16, tag="qt")
            KT = ipool.tile([128, 3 * HL * S], DT.bfloat16, tag="kt")
            VA = ipool.tile([128, B * HL * NB * D], DT.bfloat16, tag="va")
            # Load plan. First QK iteration only reads kt[:, :128] and
            # qt[:, :512] for the 3 pivot batches, so thin fast-path slices
            # go FIRST on the sync ring (~0.6 MB), then kt remainders (needed
            # from jb=2 on). Everything else goes on the scalar ring ordered
            # by first use (v hl0 -> qt rest hl0 -> all of hl1). The gpsimd
            # ring is kept clear for the steady-state T1 accumulate DMAs.
            def off_(bb, hl):
                return (bb * HL + hl) * S

            for bb in range(3):
                nc.sync.dma_start(
                    out=KT[:, off_(bb, 0) : off_(bb, 0) + 256],
                    in_=kt[bb, 0, :, 0:256],
                )
            for bb in range(3):
                nc.sync.dma_start(
                    out=QT[:, off_(bb, 0) : off_(bb, 0) + 512],
                    in_=qt[bb, 0, :, 0:512],
                )
            for bb in range(3):
                nc.sync.dma_start(
                    out=KT[:, off_(bb, 0) + 256 : off_(bb, 0) + S],
                    in_=kt[bb, 0, :, 256:S],
                )
            for b in range(B):
                voff = (b * HL + 0) * NB * D
                nc.scalar.dma_start(
                    out=VA[:, voff : voff + NB * D], in_=vv[b, 0]
                )
            for bb in range(3):
                nc.scalar.dma_start(
                    out=QT[:, off_(bb, 0) + 512 : off_(bb, 0) + S],
                    in_=qt[bb, 0, :, 512:S],
                )
            for bb in range(3):
                nc.scalar.dma_start(
                    out=KT[:, off_(bb, 1) : off_(bb, 1) + S], in_=kt[bb, 1]
                )
                nc.scalar.dma_start(
                    out=QT[:, off_(bb, 1) : off_(bb, 1) + S], in_=qt[bb, 1]
                )
            for b in range(B):
                voff = (b * HL + 1) * NB * D
                nc.scalar.dma_start(
                    out=VA[:, voff : voff + NB * D], in_=vv[b, 1]
                )

            # Software-pipelined flat loop over all 128 iterations with stage
            # lags, so each engine's strict-FIFO queue always has ready work
            # at its head:
            #   slot n:   QK(n) + exp(n) + T1-SWDGE(n) [even n]
            #   slot n+1: T1-DVE(n) [odd n]
            #   slot n+2: recip(n)
            #   slot n+3: W(n) + PV(n) (+ po evacuation at chunk end)
            NITER = HL * NI * NB

            def coords(n):
                hl = n // (NI * NB)
                r = n % (NI * NB)
                return hl, r // NB, r % NB

            state = {}
            pos = {}
            for slot in range(NITER + 3):
                if slot < NITER:
                    hl, ic, jb = coords(slot)
                    if jb == 0:
                        pos[(hl, ic)] = [
                            opsum.tile(
                                [128, 512], DT.float32, tag=f"po{p}", name=f"po{p}"
                            )
                            for p in range(2)
                        ]
                    # [128, 3*512] = 3 banks holding g_1|g_2|g_3;
                    # bufs=2 double-buffers the QK->exp handoff
                    qk = qkpool.tile([128, 1536], DT.float32, tag="qk")
                    for bb in range(3):
                        off = (bb * HL + hl) * S
                        nc.tensor.matmul(
                            qk[:, bb * 512 : (bb + 1) * 512],
                            lhsT=KT[:, off + jb * 128 : off + jb * 128 + 128],
                            rhs=QT[:, off + ic * 512 : off + ic * 512 + 512],
                            start=True,
                            stop=True,
                        )
                    E = wpool.tile([128, 1536], DT.bfloat16, tag="E")
                    nc.scalar.activation(E, qk, AF.Exp, scale=0.25)
                    T1 = wpool.tile([128, 512], DT.bfloat16, tag="T1")
                    if slot % 2 == 0:
                        # T1 = E1 + E2 off the pacing engine: SWDGE copy +
                        # CCE-accumulate pair on the gpsimd DMA ring
                        nc.gpsimd.dma_start(out=T1, in_=E[:, 0:512])
                        nc.gpsimd.dma_start(
                            out=T1, in_=E[:, 512:1024], accum_op=ALU.add
                        )
                    state[slot] = (E, T1)
                if 1 <= slot and (slot - 1) % 2 == 1 and slot - 1 < NITER:
                    E, T1 = state[slot - 1]
                    nc.vector.tensor_add(T1, E[:, 0:512], E[:, 512:1024])
                if 2 <= slot and slot - 2 < NITER:
                    E, T1 = state[slot - 2]
                    # r = 1/(1 + T1 + E3) in ONE fused DVE pass
                    rb = wpool.tile([128, 512], DT.bfloat16, tag="rb")
                    nc.vector._custom_dve(
                        recip_op,
                        out=rb,
                        in0=T1,
                        in1=E[:, 1024:1536],
                        s0=_RC0,
                        s1=_RC1,
                    )
                    state[slot - 2] = (E, T1, rb)
                if 3 <= slot and slot - 3 < NITER:
                    n = slot - 3
                    hl, ic, jb = coords(n)
                    E, T1, rb = state.pop(n)
                    W = wpool.tile([128, 1536], DT.bfloat16, tag="W")
                    nc.vector.tensor_mul(
                        W.rearrange("q (b n) -> q b n", b=3),
                        E.rearrange("q (b n) -> q b n", b=3),
                        rb.unsqueeze(1).broadcast_to([128, 3, 512]),
                    )
                    # PV rhs per batch: b0 -> rb (w_0 = r), b1..b3 -> W
                    rhss = [rb, W[:, 0:512], W[:, 512:1024], W[:, 1024:1536]]
                    po = pos[(hl, ic)]
                    for p in range(2):
                        for half in range(2):
                            b = 2 * p + half
                            voff = (b * HL + hl) * NB * D + jb * D
                            nc.tensor.matmul(
                                po[p][64 * half : 64 * (half + 1), :],
                                lhsT=VA[:, voff : voff + D],
                                rhs=rhss[b],
                                start=(jb == 0),
                                stop=(jb == NB - 1),
                                tile_position=(0, 64 * half),
                            )
                    if jb == NB - 1:
                        for p in range(2):
                            # ACT copy per batch-pair (ScalarE reads PSUM
                            # fast), then two partition-sliced DMAs
                            osb = opool.tile([128, 512], DT.float32, tag="osb")
                            nc.scalar.copy(osb, po[p])
                            for half in range(2):
                                b = 2 * p + half
                                nc.sync.dma_start(
                                    out=out[b, hl, :, ic * 512 : (ic + 1) * 512],
                                    in_=osb[64 * half : 64 * (half + 1), :],
                                )
                        del pos[(hl, ic)]

    # populate .instr bytes for InstISA subclasses (InstCustomDveAnt) — raw
    # Bass skips this pass and walrus then fails with "ISA wrong length"
    from concourse.library_overlay import lower_extended_insts

    lower_extended_insts(nc)
    return nc


def _patch_bir_waits(bir_json: bytes) -> bytes:
    """This walrus build only accepts 1 sync wait per instruction (2 for
    DMACopy); Tile emits more. Legalize:
      1. merge duplicate-semaphore waits (keep max threshold),
      2. drop waits that are transitively implied (vector-clock replay over
         the straight-line program: in-order completion per engine, FIFO per
         DMA queue, and the knowledge a producer had when it bumped a sem),
      3. split any residual multi-wait onto injected EventSemaphore
         instructions on the same engine right before the instruction.
    Only monotonic sem-inc/sem-ge-imm semaphores participate in (2); barrier
    sems (dec/eq) are left untouched."""
    import json
    from collections import defaultdict

    bir = json.loads(bir_json)

    for fn in bir["functions"]:
        insts = []
        for bb in fn["blocks"]:
            for inst in bb.get("instructions", []):
                insts.append(inst)

        # classify sems: monotonic = all updates are positive sem-inc and
        # all waits are sem-ge-imm
        bad_sems = set()
        for inst in insts:
            si = inst.get("sync_info") or {}
            for u in si.get("on_update") or []:
                if u.get("update_mode") != "sem-inc" or u.get("update_value", 0) <= 0:
                    bad_sems.add(u["id"])
            for w in si.get("on_wait") or []:
                if w.get("wait_mode") != "sem-ge-imm":
                    bad_sems.add(w["id"])

        # proc of an instruction: its engine stream, except DMACopy whose
        # completion (and sem update) is FIFO per DMA queue, keyed by the
        # sem it updates.
        def proc_of(inst):
            if inst.get("opcode") == "DMACopy":
                si = inst.get("sync_info") or {}
                ups = si.get("on_update") or []
                if ups:
                    return ("dma", ups[0]["id"])
            return ("eng", inst.get("engine"))

        sem_val = defaultdict(int)          # current cumulative value per sem
        producers = defaultdict(list)       # sem -> [(value_after, CK dict)]
        know = defaultdict(dict)            # proc -> {sem: guaranteed min}

        def join(dst, src):
            for s, v in src.items():
                if dst.get(s, 0) < v:
                    dst[s] = v

        out_blocks = {id(bb): [] for bb in fn["blocks"]}
        inj = 0
        for bb in fn["blocks"]:
            new_list = []
            for inst in bb.get("instructions", []):
                p = proc_of(inst)
                eng_p = ("eng", inst.get("engine"))
                # waits on a DMACopy are enforced by the DGE queue (FIFO per
                # queue), not the issuing engine — track knowledge per queue
                kp = p if p[0] == "dma" else eng_p
                si = inst.get("sync_info") or {}
                waits = si.get("on_wait") or []
                # merge duplicate sems
                merged = {}
                for w in waits:
                    k = w["id"]
                    if k not in merged or w.get("wait_value", 0) > merged[k].get(
                        "wait_value", 0
                    ):
                        merged[k] = w
                waits = list(merged.values())
                kept = []
                for w in waits:
                    s, v = w["id"], w.get("wait_value", 0)
                    if s in bad_sems:
                        kept.append(w)
                        continue
                    if know[kp].get(s, 0) >= v:
                        continue  # redundant
                    kept.append(w)
                    know[kp][s] = max(know[kp].get(s, 0), v)
                    # transitive knowledge from the producer that reached v
                    for val_after, ck in producers[s]:
                        if val_after >= v:
                            join(know[kp], ck)
                            break
                # split if too many waits remain
                budget = 1
                while len(kept) > budget:
                    w = kept.pop(0)
                    inj += 1
                    new_list.append(
                        {
                            "debug": inst.get("debug", 0),
                            "engine": inst.get("engine"),
                            "ins": [],
                            "name": f"WS-{inj}-{inst.get('name')}",
                            "opcode": "EventSemaphore",
                            "outs": [],
                            "sync_info": {"on_update": [], "on_wait": [w]},
                        }
                    )
                si["on_wait"] = kept
                inst["sync_info"] = si
                new_list.append(inst)
                # apply this instruction's updates for downstream knowledge
                ups = si.get("on_update") or []
                ck = None
                for u in ups:
                    s = u["id"]
                    if s in bad_sems:
                        continue
                    sem_val[s] += u.get("update_value", 0)
                    if ck is None:
                        # completion knowledge: what this proc knew here
                        # (for DMA: queue knowledge + engine state at issue)
                        ck = dict(know[kp])
                        if p[0] == "dma":
                            join(ck, know[eng_p])
                    ck[s] = sem_val[s]
                    producers[s].append((sem_val[s], ck))
                # a proc knows its own sems' values after completion
                if p[0] == "eng":
                    for u in ups:
                        if u["id"] not in bad_sems:
                            know[eng_p][u["id"]] = sem_val[u["id"]]
            out_blocks[id(bb)] = new_list
        for bb in fn["blocks"]:
            bb["instructions"] = out_blocks[id(bb)]
    return json.dumps(bir).encode()


_PATCHED = False


def _install_bir_patch():
    global _PATCHED
    if _PATCHED:
        return
    import concourse.bass2jax as bass2jax
    from concourse import bass_utils as _bu

    orig = _bu.compile_bir_kernel

    def patched(bir_json, tmpdir, neff_name="file.neff"):
        try:
            return orig(_patch_bir_waits(bir_json), tmpdir, neff_name)
        except BaseException:
            import traceback

            traceback.print_exc()
            raise

    bass2jax.compile_bir_kernel = patched
    # keep profile artifacts local — no bucket in this environment
    _bu.upload_artifacts = lambda tmpdir: str(tmpdir)
    _PATCHED = True


def _install_ntff_shim():
    """run_bass_kernel_spmd(trace=True) under axon needs
    antenv.axon_hooks.get_axon_ntff_profile_hook; the module isn't staged in
    this image, but libaxon_pjrt.so exposes the profile C ABI — recreate the
    shim (same recipe as trn_agent_boot)."""
    import sys as _sys

    if "antenv.axon_hooks" in _sys.modules:
        return
    import contextlib
    import ctypes
    import types

    import antenv  # noqa: F401

    so_path = "/opt/axon/libaxon_pjrt.so"
    hook = None
    try:
        lib = ctypes.CDLL(so_path)
        if hasattr(lib, "axon_start_nrt_profile"):
            lib.axon_start_nrt_profile.argtypes = [
                ctypes.POINTER(ctypes.c_int64),
                ctypes.c_size_t,
            ]
            lib.axon_start_nrt_profile.restype = ctypes.c_int64
            lib.axon_stop_nrt_profile.argtypes = [ctypes.c_char_p]
            lib.axon_stop_nrt_profile.restype = ctypes.c_int64

            @contextlib.contextmanager
            def hook(output_dir, device_ids):
                import jax

                jax.devices()
                if device_ids:
                    ids = (ctypes.c_int64 * len(device_ids))(*device_ids)
                    rc = lib.axon_start_nrt_profile(ids, len(device_ids))
                else:
                    rc = lib.axon_start_nrt_profile(None, 0)
                if rc != 0:
                    raise RuntimeError(f"axon_start_nrt_profile rc={rc}")
                try:
                    yield
                finally:
                    n = lib.axon_stop_nrt_profile(str(output_dir).encode())
                    print(
                        f"ntff profile: {n} file(s) -> {output_dir}",
                        file=_sys.stderr,
                    )
    except OSError:
        pass

    mod = types.ModuleType("antenv.axon_hooks")
    mod.get_axon_ntff_profile_hook = lambda: hook
    mod.set_axon_ntff_profile_hook = lambda h: None
    _sys.modules["antenv.axon_hooks"] = mod
    import antenv as _ae

    _ae.axon_hooks = mod


def kernel(query, key, value, mask=None):
    global _NC, LAST_EXEC_NS, LAST_RESULTS
    from concourse.bass_utils import run_bass_kernel_spmd

    _install_bir_patch()
    if TRACE:
        _install_ntff_shim()

    query = np.asarray(query, dtype=np.float32)
    key = np.asarray(key, dtype=np.float32)
    value = np.asarray(value, dtype=np.float32)

    if _NC is None:
        _NC = _build_nc()
    nc = _NC

    bf16 = ml_dtypes.bfloat16

    def pack_pivot(x, negate_base):
        # [B, HL, S, D] -> [B, HL, D, S]; stack [x_b^T ; (+-)x_0^T] on the
        # partition axis for b = 1..3 -> [3, HL, 128, S]
        xt = x.transpose(0, 1, 3, 2)  # [B, HL, D, S]
        base = -xt[0] if negate_base else xt[0]  # [HL, D, S]
        stk = np.stack(
            [np.concatenate([xt[b], base], axis=1) for b in (1, 2, 3)], axis=0
        )
        return np.ascontiguousarray(stk).astype(bf16)

    in_maps = []
    for c in range(NCORES):
        hs = slice(HL * c, HL * (c + 1))
        qt = pack_pivot(query[:, hs], negate_base=True)
        kt = pack_pivot(key[:, hs], negate_base=False)
        # V swizzle: [B,HL,S,D] -> [B,HL,128,NB*D] with S = NB blocks of 128
        # rows, so the device sees partition-major contiguous loads
        vc = (
            value[:, hs]
            .reshape(B, HL, NB, 128, D)
            .transpose(0, 1, 3, 2, 4)
            .reshape(B, HL, 128, NB * D)
        )
        vc = np.ascontiguousarray(vc).astype(bf16)
        in_maps.append({"qt": qt, "kt": kt, "v": vc})

    res = run_bass_kernel_spmd(
        nc, in_maps, core_ids=list(range(NCORES)), trace=TRACE
    )
    LAST_RESULTS = res
    LAST_EXEC_NS = getattr(res, "exec_time_ns", None)

    full = np.empty((B, H, S, D), dtype=np.float32)
    for c in range(NCORES):
        o = np.asarray(res.results[c]["out"])  # [B, HL, D, S]
        full[:, HL * c : HL * (c + 1)] = o.transpose(0, 1, 3, 2)
    return full


# revision 21
# speedup vs baseline: 1.2100x; 1.2100x over previous
"""Trainium2 Bass kernel for nn_Attention_10711648436709.

Math (faithful to reference):
    h = einsum('bhik,bhjk->bhij', Q, K) / sqrt(H)     # scale = sqrt(16) = 4
    w = softmax(h, axis=0)                            # over the BATCH axis (B=4)
    out = einsum('bhij,bhjv->bhiv', w, V)
    (mask is a no-op in the reference)

Sharding: head-parallel across 8 cores (16 heads -> 2 heads/core).
Softmax over batch stays core-local => communication-free.

Per-core layout trick: compute transposed scores S^T[j, i] so that
 - QK:  lhsT = K^T[d, j-block]  rhs = Q^T[d, i-chunk]   (host pre-transposes Q,K)
 - PV:  lhsT = V[j-block, v]    rhs = W[j, i-chunk]     (V in natural layout)
 - output accumulates as out^T[v, i] in PSUM; host transposes back.

Batch-0-pivot softmax: g_b = h_b - h_0 (b=1..3) computed by ONE full-K=128
matmul each (lhsT = [K_b^T ; K_0^T], rhs = [Q_b^T ; -Q_0^T], host packs).
Then w_b = E_b * r with E_b = e^{g_b/4}, r = 1/(1 + E_1 + E_2 + E_3), and
w_0 = r.  The whole denominator+reciprocal runs as ONE custom-DVE op
(r = 1/(1 + in0 + in1), BITWISE_NOT exponent-flip seed + 1 Newton pass,
max rel err ~2e-3) instead of the previous 5-op Newton / ACT Ln-Exp chain.
"""

import sys
import os

for p in ("/opt/trn_rl_repo",):
    if p not in sys.path:
        sys.path.insert(0, p)

import numpy as np
import ml_dtypes

B, H, S, D = 4, 16, 2048, 64
NCORES = 8
HL = H // NCORES          # 2 heads per core
NB = S // 128             # 16 j-blocks
NI = S // 512             # 4 i-chunks

TRACE = False
LAST_EXEC_NS = None
LAST_RESULTS = None

_NC = None
_RECIP_OP = None

# Chebyshev-minimax seed constants from RECIP_APPROX_FAST_CONSTS
_RC0 = -0.23549792
_RC1 = 2.0017324


def _register_recip1p():
    """Register the fused custom-DVE op  out = 1/(1 + in0 + in1).

    d = in0 + in1 + 1; seed = bitcast(~bits(d)) * c0  (exponent-flip trick,
    d > 1 always so the seed interval [-4.5,-4] for d*~d holds); one inline
    Newton pass y0*(c1 - d*y0).  7 ALU stages (<= 8 budget)."""
    global _RECIP_OP
    if _RECIP_OP is not None:
        return _RECIP_OP
    import concourse.dve_ops as dvo
    from concourse.dve_spec import (
        Spec,
        Src0,
        Src1,
        C0,
        C1,
        One,
        Bin,
        AluOp,
        lower,
        _has_src1 as has_src1,
    )
    from concourse.dve_uop import DveOpSpec

    NAME = "RECIP1P_ANT"
    if NAME in dvo._SUB_OPCODE_FOR_NAME:
        _RECIP_OP = next(o for o in dvo.OPS if o.name == NAME)
        return _RECIP_OP

    d = (Src0 + Src1) + One
    nx = Bin(AluOp.BITWISE_NOT, d, d)
    y0 = nx * C0
    y1 = y0 * (C1 - d * y0)

    def _ref(in0, in1, s0, s1, imm2):
        dd = in0.astype(np.float32) + in1.astype(np.float32) + np.float32(1.0)
        nxx = (~dd.view(np.int32)).view(np.float32)
        yy0 = nxx * np.float32(s0)
        return yy0 * (np.float32(s1) - dd * yy0)

    spec = Spec(body=y1, reference=_ref)
    row = max(dvo._SUB_OPCODE_FOR_NAME.values()) + 1
    assert row < 0x20, "custom-DVE opcode rows exhausted"
    shas = {}
    for ver in ("v3", "v4"):
        try:
            uops = lower(spec, ver=ver)
            shas[ver] = DveOpSpec(
                name=NAME, opcode=row, uops=uops, rd1_en=has_src1(spec)
            ).sha(ver)
        except Exception:
            pass
    op = dvo.DveOp(NAME, spec, subdim=False, uops_sha=shas)
    dvo.OPS.append(op)
    dvo.CUSTOM_DVE_SPECS[NAME] = spec
    dvo._SUB_OPCODE_FOR_NAME[NAME] = row
    _RECIP_OP = op
    return op


def _build_nc():
    import concourse.bass as bass
    import concourse.mybir as mybir
    import concourse.tile as tile

    DT = mybir.dt
    AF = mybir.ActivationFunctionType

    recip_op = _register_recip1p()

    nc = bass.Bass()
    ALU = mybir.AluOpType
    qt = nc.declare_dram_parameter("qt", [3, HL, 128, S], DT.bfloat16, isOutput=False)
    kt = nc.declare_dram_parameter("kt", [3, HL, 128, S], DT.bfloat16, isOutput=False)
    # host pre-swizzles V to [128, NB*D] per (b,hl) so the load is contiguous
    vv = nc.declare_dram_parameter(
        "v", [B, HL, 128, NB * D], DT.bfloat16, isOutput=False
    )
    out = nc.declare_dram_parameter("out", [B, HL, D, S], DT.float32, isOutput=True)

    with tile.TileContext(nc) as tc:
        with (
            tc.tile_pool(name="inputs", bufs=1) as ipool,
            tc.tile_pool(name="work", bufs=6) as wpool,
            tc.tile_pool(name="outsb", bufs=4) as opool,
            tc.tile_pool(name="qkps", bufs=2, space="PSUM") as qkpool,
            tc.tile_pool(name="ops", bufs=1, space="PSUM") as opsum,
        ):
            QT = ipool.tile([128, 3 * HL * S], DT.bfloat16, tag="qt")
            KT = ipool.tile([128, 3 * HL * S], DT.bfloat16, tag="kt")
            VA = ipool.tile([128, B * HL * NB * D], DT.bfloat16, tag="va")
            # Load plan. First QK iteration only reads kt[:, :128] and
            # qt[:, :512] for the 3 pivot batches, so thin fast-path slices
            # go FIRST on the sync ring (~0.6 MB), then kt remainders (needed
            # from jb=2 on). Everything else goes on the scalar ring ordered
            # by first use (v hl0 -> qt rest hl0 -> all of hl1). The gpsimd
            # ring is kept clear for the steady-state T1 accumulate DMAs.
            def off_(bb, hl):
                return (bb * HL + hl) * S

            for bb in range(3):
                nc.sync.dma_start(
                    out=KT[:, off_(bb, 0) : off_(bb, 0) + 256],
                    in_=kt[bb, 0, :, 0:256],
                )
            for bb in range(3):
                nc.sync.dma_start(
                    out=QT[:, off_(bb, 0) : off_(bb, 0) + 512],
                    in_=qt[bb, 0, :, 0:512],
                )
            for bb in range(3):
                nc.sync.dma_start(
                    out=KT[:, off_(bb, 0) + 256 : off_(bb, 0) + S],
                    in_=kt[bb, 0, :, 256:S],
                )
            for b in range(B):
                voff = (b * HL + 0) * NB * D
                nc.scalar.dma_start(
                    out=VA[:, voff : voff + NB * D], in_=vv[b, 0]
                )
            for bb in range(3):
                nc.scalar.dma_start(
                    out=QT[:, off_(bb, 0) + 512 : off_(bb, 0) + S],
                    in_=qt[bb, 0, :, 512:S],
                )
            for bb in range(3):
                nc.scalar.dma_start(
                    out=KT[:, off_(bb, 1) : off_(bb, 1) + S], in_=kt[bb, 1]
                )
                nc.scalar.dma_start(
                    out=QT[:, off_(bb, 1) : off_(bb, 1) + S], in_=qt[bb, 1]
                )
            for b in range(B):
                voff = (b * HL + 1) * NB * D
                nc.scalar.dma_start(
                    out=VA[:, voff : voff + NB * D], in_=vv[b, 1]
                )

            for hl in range(HL):
                for ic in range(NI):
                    po = [
                        opsum.tile(
                            [128, 512], DT.float32, tag=f"po{p}", name=f"po{p}"
                        )
                        for p in range(2)
                    ]
                    for jb in range(NB):
                        # [128, 3*512] = 3 banks holding g_1|g_2|g_3;
                        # bufs=2 double-buffers the QK->exp handoff
                        qk = qkpool.tile([128, 1536], DT.float32, tag="qk")
                        for bb in range(3):
                            off = (bb * HL + hl) * S
                            nc.tensor.matmul(
                                qk[:, bb * 512 : (bb + 1) * 512],
                                lhsT=KT[:, off + jb * 128 : off + jb * 128 + 128],
                                rhs=QT[:, off + ic * 512 : off + ic * 512 + 512],
                                start=True,
                                stop=True,
                            )
                        E = wpool.tile([128, 1536], DT.bfloat16, tag="E")
                        nc.scalar.activation(E, qk, AF.Exp, scale=0.25)
                        T1 = wpool.tile([128, 512], DT.bfloat16, tag="T1")
                        nc.vector.tensor_add(T1, E[:, 0:512], E[:, 512:1024])
                        # r = 1/(1 + T1 + E3) in ONE fused DVE pass
                        rb = wpool.tile([128, 512], DT.bfloat16, tag="rb")
                        nc.vector._custom_dve(
                            recip_op,
                            out=rb,
                            in0=T1,
                            in1=E[:, 1024:1536],
                            s0=_RC0,
                            s1=_RC1,
                        )
                        W = wpool.tile([128, 1536], DT.bfloat16, tag="W")
                        nc.vector.tensor_mul(
                            W.rearrange("q (b n) -> q b n", b=3),
                            E.rearrange("q (b n) -> q b n", b=3),
                            rb.unsqueeze(1).broadcast_to([128, 3, 512]),
                        )
                        # PV rhs per batch: b0 -> rb (w_0 = r), b1..b3 -> W
                        rhss = [rb, W[:, 0:512], W[:, 512:1024], W[:, 1024:1536]]
                        for p in range(2):
                            for half in range(2):
                                b = 2 * p + half
                                voff = (b * HL + hl) * NB * D + jb * D
                                nc.tensor.matmul(
                                    po[p][64 * half : 64 * (half + 1), :],
                                    lhsT=VA[:, voff : voff + D],
                                    rhs=rhss[b],
                                    start=(jb == 0),
                                    stop=(jb == NB - 1),
                                    tile_position=(0, 64 * half),
                                )
                    for p in range(2):
                        # ACT copy per batch-pair (ScalarE reads PSUM fast),
                        # then two partition-sliced DMAs to DRAM
                        osb = opool.tile([128, 512], DT.float32, tag="osb")
                        nc.scalar.copy(osb, po[p])
                        for half in range(2):
                            b = 2 * p + half
                            nc.sync.dma_start(
                                out=out[b, hl, :, ic * 512 : (ic + 1) * 512],
                                in_=osb[64 * half : 64 * (half + 1), :],
                            )

    # populate .instr bytes for InstISA subclasses (InstCustomDveAnt) — raw
    # Bass skips this pass and walrus then fails with "ISA wrong length"
    from concourse.library_overlay import lower_extended_insts

    lower_extended_insts(nc)
    return nc


def _patch_bir_waits(bir_json: bytes) -> bytes:
    """This walrus build only accepts 1 sync wait per instruction (2 for
    DMACopy); Tile emits more. Legalize:
      1. merge duplicate-semaphore waits (keep max threshold),
      2. drop waits that are transitively implied (vector-clock replay over
         the straight-line program: in-order completion per engine, FIFO per
         DMA queue, and the knowledge a producer had when it bumped a sem),
      3. split any residual multi-wait onto injected EventSemaphore
         instructions on the same engine right before the instruction.
    Only monotonic sem-inc/sem-ge-imm semaphores participate in (2); barrier
    sems (dec/eq) are left untouched."""
    import json
    from collections import defaultdict

    bir = json.loads(bir_json)

    for fn in bir["functions"]:
        insts = []
        for bb in fn["blocks"]:
            for inst in bb.get("instructions", []):
                insts.append(inst)

        # classify sems: monotonic = all updates are positive sem-inc and
        # all waits are sem-ge-imm
        bad_sems = set()
        for inst in insts:
            si = inst.get("sync_info") or {}
            for u in si.get("on_update") or []:
                if u.get("update_mode") != "sem-inc" or u.get("update_value", 0) <= 0:
                    bad_sems.add(u["id"])
            for w in si.get("on_wait") or []:
                if w.get("wait_mode") != "sem-ge-imm":
                    bad_sems.add(w["id"])

        # proc of an instruction: its engine stream, except DMACopy whose
        # completion (and sem update) is FIFO per DMA queue, keyed by the
        # sem it updates.
        def proc_of(inst):
            if inst.get("opcode") == "DMACopy":
                si = inst.get("sync_info") or {}
                ups = si.get("on_update") or []
                if ups:
                    return ("dma", ups[0]["id"])
            return ("eng", inst.get("engine"))

        sem_val = defaultdict(int)          # current cumulative value per sem
        producers = defaultdict(list)       # sem -> [(value_after, CK dict)]
        know = defaultdict(dict)            # proc -> {sem: guaranteed min}

        def join(dst, src):
            for s, v in src.items():
                if dst.get(s, 0) < v:
                    dst[s] = v

        out_blocks = {id(bb): [] for bb in fn["blocks"]}
        inj = 0
        for bb in fn["blocks"]:
            new_list = []
            for inst in bb.get("instructions", []):
                p = proc_of(inst)
                eng_p = ("eng", inst.get("engine"))
                # waits on a DMACopy are enforced by the DGE queue (FIFO per
                # queue), not the issuing engine — track knowledge per queue
                kp = p if p[0] == "dma" else eng_p
                si = inst.get("sync_info") or {}
                waits = si.get("on_wait") or []
                # merge duplicate sems
                merged = {}
                for w in waits:
                    k = w["id"]
                    if k not in merged or w.get("wait_value", 0) > merged[k].get(
                        "wait_value", 0
                    ):
                        merged[k] = w
                waits = list(merged.values())
                kept = []
                for w in waits:
                    s, v = w["id"], w.get("wait_value", 0)
                    if s in bad_sems:
                        kept.append(w)
                        continue
                    if know[kp].get(s, 0) >= v:
                        continue  # redundant
                    kept.append(w)
                    know[kp][s] = max(know[kp].get(s, 0), v)
                    # transitive knowledge from the producer that reached v
                    for val_after, ck in producers[s]:
                        if val_after >= v:
                            join(know[kp], ck)
                            break
                # split if too many waits remain
                budget = 1
                while len(kept) > budget:
                    w = kept.pop(0)
                    inj += 1
                    new_list.append(
                        {
                            "debug": inst.get("debug", 0),
                            "engine": inst.get("engine"),
                            "ins": [],
                            "name": f"WS-{inj}-{inst.get('name')}",
                            "opcode": "EventSemaphore",
                            "outs": [],
                            "sync_info": {"on_update": [], "on_wait": [w]},
                        }
                    )
                si["on_wait"] = kept
                inst["sync_info"] = si
                new_list.append(inst)
                # apply this instruction's updates for downstream knowledge
                ups = si.get("on_update") or []
                ck = None
                for u in ups:
                    s = u["id"]
                    if s in bad_sems:
                        continue
                    sem_val[s] += u.get("update_value", 0)
                    if ck is None:
                        # completion knowledge: what this proc knew here
                        # (for DMA: queue knowledge + engine state at issue)
                        ck = dict(know[kp])
                        if p[0] == "dma":
                            join(ck, know[eng_p])
                    ck[s] = sem_val[s]
                    producers[s].append((sem_val[s], ck))
                # a proc knows its own sems' values after completion
                if p[0] == "eng":
                    for u in ups:
                        if u["id"] not in bad_sems:
                            know[eng_p][u["id"]] = sem_val[u["id"]]
            out_blocks[id(bb)] = new_list
        for bb in fn["blocks"]:
            bb["instructions"] = out_blocks[id(bb)]
    return json.dumps(bir).encode()


_PATCHED = False


def _install_bir_patch():
    global _PATCHED
    if _PATCHED:
        return
    import concourse.bass2jax as bass2jax
    from concourse import bass_utils as _bu

    orig = _bu.compile_bir_kernel

    def patched(bir_json, tmpdir, neff_name="file.neff"):
        try:
            return orig(_patch_bir_waits(bir_json), tmpdir, neff_name)
        except BaseException:
            import traceback

            traceback.print_exc()
            raise

    bass2jax.compile_bir_kernel = patched
    # keep profile artifacts local — no bucket in this environment
    _bu.upload_artifacts = lambda tmpdir: str(tmpdir)
    _PATCHED = True


def _install_ntff_shim():
    """run_bass_kernel_spmd(trace=True) under axon needs
    antenv.axon_hooks.get_axon_ntff_profile_hook; the module isn't staged in
    this image, but libaxon_pjrt.so exposes the profile C ABI — recreate the
    shim (same recipe as trn_agent_boot)."""
    import sys as _sys

    if "antenv.axon_hooks" in _sys.modules:
        return
    import contextlib
    import ctypes
    import types

    import antenv  # noqa: F401

    so_path = "/opt/axon/libaxon_pjrt.so"
    hook = None
    try:
        lib = ctypes.CDLL(so_path)
        if hasattr(lib, "axon_start_nrt_profile"):
            lib.axon_start_nrt_profile.argtypes = [
                ctypes.POINTER(ctypes.c_int64),
                ctypes.c_size_t,
            ]
            lib.axon_start_nrt_profile.restype = ctypes.c_int64
            lib.axon_stop_nrt_profile.argtypes = [ctypes.c_char_p]
            lib.axon_stop_nrt_profile.restype = ctypes.c_int64

            @contextlib.contextmanager
            def hook(output_dir, device_ids):
                import jax

                jax.devices()
                if device_ids:
                    ids = (ctypes.c_int64 * len(device_ids))(*device_ids)
                    rc = lib.axon_start_nrt_profile(ids, len(device_ids))
                else:
                    rc = lib.axon_start_nrt_profile(None, 0)
                if rc != 0:
                    raise RuntimeError(f"axon_start_nrt_profile rc={rc}")
                try:
                    yield
                finally:
                    n = lib.axon_stop_nrt_profile(str(output_dir).encode())
                    print(
                        f"ntff profile: {n} file(s) -> {output_dir}",
                        file=_sys.stderr,
                    )
    except OSError:
        pass

    mod = types.ModuleType("antenv.axon_hooks")
    mod.get_axon_ntff_profile_hook = lambda: hook
    mod.set_axon_ntff_profile_hook = lambda h: None
    _sys.modules["antenv.axon_hooks"] = mod
    import antenv as _ae

    _ae.axon_hooks = mod


def kernel(query, key, value, mask=None):
    global _NC, LAST_EXEC_NS, LAST_RESULTS
    from concourse.bass_utils import run_bass_kernel_spmd

    _install_bir_patch()
    if TRACE:
        _install_ntff_shim()

    query = np.asarray(query, dtype=np.float32)
    key = np.asarray(key, dtype=np.float32)
    value = np.asarray(value, dtype=np.float32)

    if _NC is None:
        _NC = _build_nc()
    nc = _NC

    bf16 = ml_dtypes.bfloat16

    def pack_pivot(x, negate_base):
        # [B, HL, S, D] -> [B, HL, D, S]; stack [x_b^T ; (+-)x_0^T] on the
        # partition axis for b = 1..3 -> [3, HL, 128, S]
        xt = x.transpose(0, 1, 3, 2)  # [B, HL, D, S]
        base = -xt[0] if negate_base else xt[0]  # [HL, D, S]
        stk = np.stack(
            [np.concatenate([xt[b], base], axis=1) for b in (1, 2, 3)], axis=0
        )
        return np.ascontiguousarray(stk).astype(bf16)

    in_maps = []
    for c in range(NCORES):
        hs = slice(HL * c, HL * (c + 1))
        qt = pack_pivot(query[:, hs], negate_base=True)
        kt = pack_pivot(key[:, hs], negate_base=False)
        # V swizzle: [B,HL,S,D] -> [B,HL,128,NB*D] with S = NB blocks of 128
        # rows, so the device sees partition-major contiguous loads
        vc = (
            value[:, hs]
            .reshape(B, HL, NB, 128, D)
            .transpose(0, 1, 3, 2, 4)
            .reshape(B, HL, 128, NB * D)
        )
        vc = np.ascontiguousarray(vc).astype(bf16)
        in_maps.append({"qt": qt, "kt": kt, "v": vc})

    res = run_bass_kernel_spmd(
        nc, in_maps, core_ids=list(range(NCORES)), trace=TRACE
    )
    LAST_RESULTS = res
    LAST_EXEC_NS = getattr(res, "exec_time_ns", None)

    full = np.empty((B, H, S, D), dtype=np.float32)
    for c in range(NCORES):
        o = np.asarray(res.results[c]["out"])  # [B, HL, D, S]
        full[:, HL * c : HL * (c + 1)] = o.transpose(0, 1, 3, 2)
    return full
